# revision 27
# baseline (speedup 1.0000x reference)
"""Per-pixel blur (BatchBlur_nopad) Trainium2 kernel.

Math: out[b,c,i,j] = sum_{kh,kw} input[b,c,i+kh,j+kw] * kernel[b,kh*19+kw,i+9,j+9]
Shapes: input [4,3,256,256] f32, kernel [4,361,256,256] f32 -> out [4,3,238,238] f32.

Sharding: 8 cores = (batch, row-half). Each core owns out[b, :, half*119:(half+1)*119, :].
Per-core pipeline (VectorE-bound):
  for kh in 0..18:
    prod[c,kw,j] = in_f16[c, i+kh, j+kw] * w_f16[kh, i, kw, j]   (fp16 TT mult @2x mode,
                     split in even/odd kw so every packed read stays 4B-aligned)
    contrib[c,j] = sum_kw prod    (mode "red": tensor_reduce @1x;
                                   mode "tree": pairwise fp16 TT adds @2x)
    acc += contrib                                               (fp32)
ScalarE casts f32->f16 (input once, w streamed per kh); DMA double-buffered.
"""

import threading

import numpy as np

import concourse.bass as bass
import concourse.bacc as bacc
import concourse.mybir as mybir
import concourse.tile as tile
from concourse.bass_utils import run_bass_kernel_spmd

B, C, H, W = 4, 3, 256, 256
L, PAD = 19, 9
Ho = Wo = H - L + 1          # 238
RPC = Ho // 2                # 119 output rows per core
IN_ROWS = RPC + L - 1        # 137 input rows per core
NCORES = 8
KE = (L + 1) // 2            # even kw taps: 0,2,..,18 -> 10
KO = L // 2                  # odd  kw taps: 1,3,..,17 -> 9

# Contrib mode, HW-measured per main-loop pass (all 8 cores in parallel):
#   "red"  (tensor_reduce @1x, strided-read penalty): 617.5 us
#   "tree" (pairwise fp16 adds @2x):                  297.7 us
#   "tree2" (kh-pairs share one tree):                ~288 us  <- production
MODE = "tree2"

f32 = mybir.dt.float32
f16 = mybir.dt.float16

_lock = threading.Lock()
_cache = {}


def _mk(t, extra_offset, dims):
    """AP over t's tensor at t.offset+extra_offset with explicit (step, count) dims."""
    return bass.AP(t.tensor, t.offset + extra_offset, [list(d) for d in dims])


def _load_input(nc, ring, in_d, in_e, in_o):
    P = RPC
    for kh in range(L):
        stage = ring.tile([P, C, W], f32, tag="stage")
        nc.sync.dma_start(out=stage, in_=in_d[:, kh : kh + P, :].transpose([1, 0, 2]))
        nc.scalar.copy(out=in_e[:, kh, :, :], in_=stage)
        nc.scalar.copy(out=in_o[:, kh, :, 0 : W - 1], in_=stage[:, :, 1:W])


def _mults(nc, in_e, in_o, w16, kh, prod):
    """The two fp16 @2x multiplies for one kh into prod [P, C, L, Wo]."""
    P = RPC
    pstep_e = in_e.ap[0][0]
    # even kw = 2m: read in_e[i, kh, c, j+2m]
    a_e = _mk(in_e, kh * C * W, [(pstep_e, P), (W, C), (2, KE), (1, Wo)])
    w_e = _mk(w16, 0, [(w16.ap[0][0], P), (0, C), (2 * Wo, KE), (1, Wo)])
    p_e = _mk(prod, 0, [(prod.ap[0][0], P), (L * Wo, C), (2 * Wo, KE), (1, Wo)])
    nc.vector.tensor_mul(p_e, a_e, w_e)
    # odd kw = 2m+1: read in_o[i, kh, c, j+2m] (in_o holds the +1 shift)
    a_o = _mk(in_o, kh * C * W, [(pstep_e, P), (W, C), (2, KO), (1, Wo)])
    w_o = _mk(w16, Wo, [(w16.ap[0][0], P), (0, C), (2 * Wo, KO), (1, Wo)])
    p_o = _mk(prod, Wo, [(prod.ap[0][0], P), (L * Wo, C), (2 * Wo, KO), (1, Wo)])
    nc.vector.tensor_mul(p_o, a_o, w_o)


def _tree_fold(nc, prodp, prod, acc, first):
    """Pairwise fp16 adds @2x folding prod [P,C,L,Wo] over kw, then acc-update."""
    P = RPC
    t8 = prodp.tile([P, C, 8, Wo], f16, tag="t8", name="t8", bufs=1)
    nc.vector.tensor_add(t8[:], prod[:, :, 0:8, :], prod[:, :, 8:16, :])
    t4 = prodp.tile([P, C, 4, Wo], f16, tag="t4", name="t4", bufs=1)
    nc.vector.tensor_add(t4[:], t8[:, :, 0:4, :], t8[:, :, 4:8, :])
    t2 = prodp.tile([P, C, 2, Wo], f16, tag="t2", name="t2", bufs=1)
    nc.vector.tensor_add(t2[:], t4[:, :, 0:2, :], t4[:, :, 2:4, :])
    t1 = prodp.tile([P, C, Wo], f16, tag="t1", name="t1", bufs=1)
    nc.vector.tensor_add(t1[:], t2[:, :, 0, :], t2[:, :, 1, :])
    ta = prodp.tile([P, C, Wo], f16, tag="ta", name="ta", bufs=1)
    nc.vector.tensor_add(ta[:], prod[:, :, 16, :], prod[:, :, 17, :])
    tb = prodp.tile([P, C, Wo], f16, tag="tb", name="tb", bufs=1)
    nc.vector.tensor_add(tb[:], t1[:], ta[:])
    tc_ = prodp.tile([P, C, Wo], f16, tag="tc", name="tc_", bufs=1)
    nc.vector.tensor_add(tc_[:], tb[:], prod[:, :, 18, :])
    if first:
        nc.vector.tensor_copy(acc[:], tc_[:])
    else:
        nc.vector.tensor_add(acc[:], acc[:], tc_[:])


def _kh_body(nc, pools, in_e, in_o, acc, w16, kh, first, mode):
    """One kh iteration: mults into prod, kw-reduction, accumulate into acc.

    mode: "red" | "tree" (production) | "multonly" | "red_noacc" (bench-only)."""
    P = RPC
    ring, prodp = pools
    pstep_e = in_e.ap[0][0]

    # All DVE work is engine-serial; single-buffered tiles cost no overlap.
    prod = prodp.tile([P, C, L, Wo], f16, tag="prod", name="prod", bufs=1)
    _mults(nc, in_e, in_o, w16, kh, prod)

    if mode == "multonly":
        return
    if mode in ("red", "red_noacc"):
        # reduce over kw (innermost AP dim), fp32 out
        red_in = _mk(prod, 0, [(prod.ap[0][0], P), (L * Wo, C), (1, Wo), (Wo, L)])
        if first and mode == "red":
            nc.vector.tensor_reduce(
                out=acc[:], in_=red_in, axis=mybir.AxisListType.X, op=mybir.AluOpType.add
            )
        else:
            contrib = ring.tile([P, C, Wo], f32, tag="contrib", name="contrib")
            nc.vector.tensor_reduce(
                out=contrib[:], in_=red_in, axis=mybir.AxisListType.X, op=mybir.AluOpType.add
            )
            if mode == "red":
                nc.vector.tensor_add(acc[:], acc[:], contrib[:])
    elif mode == "tree":
        _tree_fold(nc, prodp, prod, acc, first)
    else:
        raise ValueError(mode)


def _emit(nc, tc, in_d, w_d, out_d, repeat=1, mode=MODE):
    P = RPC
    with (
        tc.tile_pool(name="persist", bufs=1) as persist,
        tc.tile_pool(name="ring", bufs=2) as ring,
        tc.tile_pool(name="prodp", bufs=2) as prodp,
    ):
        in_e = persist.tile([P, L, C, W], f16)      # in_e[i,kh,c,t] = in[c, i+kh, t]
        in_o = persist.tile([P, L, C, W], f16)      # in_o[i,kh,c,t] = in[c, i+kh, t+1]
        acc = persist.tile([P, C, Wo], f32)
        w16_rep = None
        if repeat > 1:
            w16_rep = persist.tile([P, L, Wo], f16)

        _load_input(nc, ring, in_d, in_e, in_o)
        if mode in ("multonly", "red_noacc"):
            nc.vector.memzero(acc[:])  # bench modes never write acc; out-DMA reads it

        def get_w16(kh, rep):
            if rep > 0:
                return w16_rep
            w32 = ring.tile([P, L, Wo], f32, tag="w32", name="w32")
            nc.sync.dma_start(out=w32, in_=w_d[kh])
            w16 = ring.tile([P, L, Wo], f16, tag="w16", name="w16")
            nc.scalar.copy(out=w16, in_=w32)
            if kh == 0 and w16_rep is not None:
                nc.scalar.copy(out=w16_rep[:], in_=w32)
            return w16

        for rep in range(repeat):
            if mode == "tree2":
                for kh0 in range(0, L, 2):
                    prod_a = prodp.tile([P, C, L, Wo], f16, tag="prodA", name="prod_a", bufs=1)
                    _mults(nc, in_e, in_o, get_w16(kh0, rep), kh0, prod_a)
                    if kh0 + 1 < L:
                        prod_b = prodp.tile(
                            [P, C, L, Wo], f16, tag="prodB", name="prod_b", bufs=1
                        )
                        _mults(nc, in_e, in_o, get_w16(kh0 + 1, rep), kh0 + 1, prod_b)
                        nc.vector.tensor_add(prod_a[:], prod_a[:], prod_b[:])
                    _tree_fold(nc, prodp, prod_a, acc, first=(kh0 == 0 and rep == 0))
            else:
                for kh in range(L):
                    _kh_body(
                        nc, (ring, prodp), in_e, in_o, acc, get_w16(kh, rep), kh,
                        first=(kh == 0 and rep == 0), mode=mode,
                    )

        nc.sync.dma_start(out=out_d.transpose([1, 0, 2]), in_=acc[:])


def _emit_hw_loop(nc, tc, in_d, w_d, out_d, n_iters, mode=MODE):
    """Timing variant: the exact production main loop repeated n_iters times in
    a hardware loop (values after iteration 0 are garbage; per-iteration
    instruction/DMA mix identical to production)."""
    P = RPC
    with (
        tc.tile_pool(name="persist", bufs=1) as persist,
        tc.tile_pool(name="ring", bufs=2) as ring,
        tc.tile_pool(name="prodp", bufs=2) as prodp,
    ):
        in_e = persist.tile([P, L, C, W], f16)
        in_o = persist.tile([P, L, C, W], f16)
        acc = persist.tile([P, C, Wo], f32)

        _load_input(nc, ring, in_d, in_e, in_o)
        if mode in ("multonly", "red_noacc"):
            nc.vector.memzero(acc[:])  # bench modes never write acc; out-DMA reads it

        def get_w16(kh):
            w32 = ring.tile([P, L, Wo], f32, tag="w32", name="w32")
            nc.sync.dma_start(out=w32, in_=w_d[kh])
            w16 = ring.tile([P, L, Wo], f16, tag="w16", name="w16")
            nc.scalar.copy(out=w16, in_=w32)
            return w16

        with tc.For_i(0, n_iters, 1):
            if mode == "tree2":
                for kh0 in range(0, L, 2):
                    prod_a = prodp.tile([P, C, L, Wo], f16, tag="prodA", name="prod_a", bufs=1)
                    _mults(nc, in_e, in_o, get_w16(kh0), kh0, prod_a)
                    if kh0 + 1 < L:
                        prod_b = prodp.tile(
                            [P, C, L, Wo], f16, tag="prodB", name="prod_b", bufs=1
                        )
                        _mults(nc, in_e, in_o, get_w16(kh0 + 1), kh0 + 1, prod_b)
                        nc.vector.tensor_add(prod_a[:], prod_a[:], prod_b[:])
                    _tree_fold(nc, prodp, prod_a, acc, first=(kh0 == 0))
            else:
                for kh in range(L):
                    _kh_body(
                        nc, (ring, prodp), in_e, in_o, acc, get_w16(kh), kh,
                        first=(kh == 0), mode=mode,
                    )

        nc.sync.dma_start(out=out_d.transpose([1, 0, 2]), in_=acc[:])


def _emit_probe(nc, tc, n_iters, probe):
    """Microbenchmark: 8 identical DVE instructions per hw-loop iteration."""
    P = RPC
    NEL = C * L * Wo  # 13566
    bf16 = mybir.dt.bfloat16
    with (
        tc.tile_pool(name="persist", bufs=1) as persist,
    ):
        dt_map = {"16": f16, "bf": bf16, "32": f32}
        a16 = persist.tile([P, NEL], f16)
        b16 = persist.tile([P, NEL], f16)
        o16 = persist.tile([P, NEL], f16)
        abf = persist.tile([P, NEL], bf16)
        bbf = persist.tile([P, NEL], bf16)
        obf = persist.tile([P, NEL], bf16)
        for t in (a16, b16, abf, bbf):
            nc.vector.memzero(t[:])

        ine = persist.tile([P, C, W], f16)   # small input plane for windowed probes
        wt = persist.tile([P, L, Wo], f16)
        nc.vector.memzero(ine[:])
        nc.vector.memzero(wt[:])

        def win_mult(suffix_dims_a, dims_w, dims_o):
            nc.vector.tensor_mul(
                _mk(o16, 0, dims_o), _mk(ine, 0, suffix_dims_a), _mk(wt, 0, dims_w)
            )

        with tc.For_i(0, n_iters, 1):
            if probe == "tiny":
                # near-empty body: measures the For_i back-edge cost
                nc.vector.tensor_copy(o16[:, :16], a16[:, :16])
            for _ in range(0 if probe == "tiny" else 8):
                if probe == "flat16":
                    nc.vector.tensor_mul(o16[:], a16[:], b16[:])
                elif probe == "flatbf":
                    nc.vector.tensor_mul(obf[:], abf[:], bbf[:])
                elif probe == "flat16_half":
                    nc.vector.tensor_mul(
                        o16[:, : NEL // 2], a16[:, : NEL // 2], b16[:, : NEL // 2]
                    )
                elif probe == "copy16":
                    nc.vector.tensor_copy(o16[:], a16[:])
                elif probe == "mult_noc":
                    # per-c windowed mult, 3-dim APs, no broadcast (KE evens only)
                    for c in range(C):
                        win_mult(
                            [(ine.ap[0][0], P), (2, KE), (1, Wo)],
                            [(wt.ap[0][0], P), (2 * Wo, KE), (1, Wo)],
                            [(o16.ap[0][0], P), (2 * Wo, KE), (1, Wo)],
                        )
                elif probe == "mult_nowin":
                    # same shape but NON-overlapping strided reads from big tiles
                    for c in range(C):
                        nc.vector.tensor_mul(
                            _mk(o16, 0, [(o16.ap[0][0], P), (2 * Wo, KE), (1, Wo)]),
                            _mk(a16, 0, [(a16.ap[0][0], P), (Wo, KE), (1, Wo)]),
                            _mk(b16, 0, [(b16.ap[0][0], P), (2 * Wo, KE), (1, Wo)]),
                        )
                elif probe == "mult_bcast":
                    # the real even-mult shape incl c-broadcast on w (4-dim)
                    nc.vector.tensor_mul(
                        _mk(o16, 0, [(o16.ap[0][0], P), (L * Wo, C), (2 * Wo, KE), (1, Wo)]),
                        _mk(ine, 0, [(ine.ap[0][0], P), (W, C), (2, KE), (1, Wo)]),
                        _mk(wt, 0, [(wt.ap[0][0], P), (0, C), (2 * Wo, KE), (1, Wo)]),
                    )
                else:
                    raise ValueError(probe)


def build_probe(probe, hw_loop):
    key = ("probe", probe, hw_loop)
    with _lock:
        if key in _cache:
            return _cache[key]
        nc = bacc.Bacc("TRN2", target_bir_lowering=False, debug=False)
        in_d = nc.dram_tensor("in_slab", [C, IN_ROWS, W], f32, kind="ExternalInput")
        w_d = nc.dram_tensor("w_slab", [L, RPC, L, Wo], f32, kind="ExternalInput")
        out_d = nc.dram_tensor("out", [C, RPC, Wo], f32, kind="ExternalOutput")
        with tile.TileContext(nc) as tc:
            with tc.tile_pool(name="io", bufs=1) as io:
                sink = io.tile([RPC, C, Wo], f32)
                nc.sync.dma_start(out=sink, in_=w_d[0][:, 0:C, :])
                _emit_probe(nc, tc, hw_loop, probe)
                nc.vector.memzero(sink[:])
                nc.sync.dma_start(out=out_d.transpose([1, 0, 2]), in_=sink[:])
        nc.compile()
        _cache[key] = nc
        return nc
    with _lock:
        if key in _cache:
            return _cache[key]
        nc = bacc.Bacc("TRN2", target_bir_lowering=False, debug=False)
        in_d = nc.dram_tensor("in_slab", [C, IN_ROWS, W], f32, kind="ExternalInput")
        w_d = nc.dram_tensor("w_slab", [L, RPC, L, Wo], f32, kind="ExternalInput")
        out_d = nc.dram_tensor("out", [C, RPC, Wo], f32, kind="ExternalOutput")
        with tile.TileContext(nc) as tc:
            if hw_loop:
                _emit_hw_loop(nc, tc, in_d.ap(), w_d.ap(), out_d.ap(), n_iters=hw_loop, mode=mode)
            else:
                _emit(nc, tc, in_d.ap(), w_d.ap(), out_d.ap(), repeat=repeat, mode=mode)
        nc.compile()
        _cache[key] = nc
        return nc


def build_program(repeat=1, hw_loop=0, mode=MODE):
    key = ("prog", repeat, hw_loop, mode)
    with _lock:
        if key in _cache:
            return _cache[key]
        nc = bacc.Bacc("TRN2", target_bir_lowering=False, debug=False)
        in_d = nc.dram_tensor("in_slab", [C, IN_ROWS, W], f32, kind="ExternalInput")
        w_d = nc.dram_tensor("w_slab", [L, RPC, L, Wo], f32, kind="ExternalInput")
        out_d = nc.dram_tensor("out", [C, RPC, Wo], f32, kind="ExternalOutput")
        with tile.TileContext(nc) as tc:
            if hw_loop:
                _emit_hw_loop(nc, tc, in_d.ap(), w_d.ap(), out_d.ap(), n_iters=hw_loop, mode=mode)
            else:
                _emit(nc, tc, in_d.ap(), w_d.ap(), out_d.ap(), repeat=repeat, mode=mode)
        nc.compile()
        _cache[key] = nc
        return nc


def make_in_maps(input, kernel):
    in_maps = []
    for core in range(NCORES):
        b, half = divmod(core, 2)
        r0 = half * RPC
        in_sl = np.ascontiguousarray(input[b, :, r0 : r0 + IN_ROWS, :], dtype=np.float32)
        kx = kernel[b, :, PAD + r0 : PAD + r0 + RPC, PAD : PAD + Wo]  # [361, 119, 238]
        w_sl = np.ascontiguousarray(
            kx.reshape(L, L, RPC, Wo).transpose(0, 2, 1, 3), dtype=np.float32
        )  # [kh, i, kw, j]
        in_maps.append({"in_slab": in_sl, "w_slab": w_sl})
    return in_maps


def gather_out(results):
    out = np.empty((B, C, Ho, Wo), dtype=np.float32)
    for core in range(NCORES):
        b, half = divmod(core, 2)
        out[b, :, half * RPC : (half + 1) * RPC, :] = results[core]["out"]
    return out


def run(input, kernel, **spmd_kwargs):
    nc = build_program()
    in_maps = make_in_maps(input, kernel)
    res = run_bass_kernel_spmd(nc, in_maps, core_ids=list(range(NCORES)), **spmd_kwargs)
    return gather_out(res.results), res


def kernel(**inputs):
    out, _ = run(np.asarray(inputs["input"]), np.asarray(inputs["kernel"]))
    return out


# revision 29
# speedup vs baseline: 1.0067x; 1.0067x over previous
"""Per-pixel blur (BatchBlur_nopad) Trainium2 kernel.

Math: out[b,c,i,j] = sum_{kh,kw} input[b,c,i+kh,j+kw] * kernel[b,kh*19+kw,i+9,j+9]
Shapes: input [4,3,256,256] f32, kernel [4,361,256,256] f32 -> out [4,3,238,238] f32.

Sharding: 8 cores = (batch, row-half). Each core owns out[b, :, half*119:(half+1)*119, :].
Per-core pipeline (VectorE-bound):
  for kh in 0..18:
    prod[c,kw,j] = in_f16[c, i+kh, j+kw] * w_f16[kh, i, kw, j]   (fp16 TT mult @2x mode,
                     split in even/odd kw so every packed read stays 4B-aligned)
    contrib[c,j] = sum_kw prod    (mode "red": tensor_reduce @1x;
                                   mode "tree": pairwise fp16 TT adds @2x)
    acc += contrib                                               (fp32)
ScalarE casts f32->f16 (input once, w streamed per kh); DMA double-buffered.
"""

import threading

import numpy as np

import concourse.bass as bass
import concourse.bacc as bacc
import concourse.mybir as mybir
import concourse.tile as tile
from concourse.bass_utils import run_bass_kernel_spmd

B, C, H, W = 4, 3, 256, 256
L, PAD = 19, 9
Ho = Wo = H - L + 1          # 238
RPC = Ho // 2                # 119 output rows per core
IN_ROWS = RPC + L - 1        # 137 input rows per core
NCORES = 8
KE = (L + 1) // 2            # even kw taps: 0,2,..,18 -> 10
KO = L // 2                  # odd  kw taps: 1,3,..,17 -> 9

# Contrib mode, per main-loop pass (all 8 cores in parallel):
#   "red"  (tensor_reduce @1x, strided-read penalty): 617.5 us (HW)
#   "tree" (pairwise fp16 adds @2x):                  297.7 us (HW)
#   "tree2" (kh-pairs share one tree):                287.9 us (model, HW-confirmed)
#   "tree4" (4-kh groups share one tree):             283.2 us (model) <- production
MODE = "tree4"

f32 = mybir.dt.float32
f16 = mybir.dt.float16

_lock = threading.Lock()
_cache = {}


def _mk(t, extra_offset, dims):
    """AP over t's tensor at t.offset+extra_offset with explicit (step, count) dims."""
    return bass.AP(t.tensor, t.offset + extra_offset, [list(d) for d in dims])


def _load_input(nc, ring, in_d, in_e, in_o):
    P = RPC
    for kh in range(L):
        stage = ring.tile([P, C, W], f32, tag="stage")
        nc.sync.dma_start(out=stage, in_=in_d[:, kh : kh + P, :].transpose([1, 0, 2]))
        nc.scalar.copy(out=in_e[:, kh, :, :], in_=stage)
        nc.scalar.copy(out=in_o[:, kh, :, 0 : W - 1], in_=stage[:, :, 1:W])


def _mults(nc, in_e, in_o, w16, kh, prod):
    """The two fp16 @2x multiplies for one kh into prod [P, C, L, Wo]."""
    P = RPC
    pstep_e = in_e.ap[0][0]
    # even kw = 2m: read in_e[i, kh, c, j+2m]
    a_e = _mk(in_e, kh * C * W, [(pstep_e, P), (W, C), (2, KE), (1, Wo)])
    w_e = _mk(w16, 0, [(w16.ap[0][0], P), (0, C), (2 * Wo, KE), (1, Wo)])
    p_e = _mk(prod, 0, [(prod.ap[0][0], P), (L * Wo, C), (2 * Wo, KE), (1, Wo)])
    nc.vector.tensor_mul(p_e, a_e, w_e)
    # odd kw = 2m+1: read in_o[i, kh, c, j+2m] (in_o holds the +1 shift)
    a_o = _mk(in_o, kh * C * W, [(pstep_e, P), (W, C), (2, KO), (1, Wo)])
    w_o = _mk(w16, Wo, [(w16.ap[0][0], P), (0, C), (2 * Wo, KO), (1, Wo)])
    p_o = _mk(prod, Wo, [(prod.ap[0][0], P), (L * Wo, C), (2 * Wo, KO), (1, Wo)])
    nc.vector.tensor_mul(p_o, a_o, w_o)


def _tree_fold(nc, prodp, prod, acc, first):
    """Pairwise fp16 adds @2x folding prod [P,C,L,Wo] over kw, then acc-update."""
    P = RPC
    t8 = prodp.tile([P, C, 8, Wo], f16, tag="t8", name="t8", bufs=1)
    nc.vector.tensor_add(t8[:], prod[:, :, 0:8, :], prod[:, :, 8:16, :])
    t4 = prodp.tile([P, C, 4, Wo], f16, tag="t4", name="t4", bufs=1)
    nc.vector.tensor_add(t4[:], t8[:, :, 0:4, :], t8[:, :, 4:8, :])
    t2 = prodp.tile([P, C, 2, Wo], f16, tag="t2", name="t2", bufs=1)
    nc.vector.tensor_add(t2[:], t4[:, :, 0:2, :], t4[:, :, 2:4, :])
    t1 = prodp.tile([P, C, Wo], f16, tag="t1", name="t1", bufs=1)
    nc.vector.tensor_add(t1[:], t2[:, :, 0, :], t2[:, :, 1, :])
    ta = prodp.tile([P, C, Wo], f16, tag="ta", name="ta", bufs=1)
    nc.vector.tensor_add(ta[:], prod[:, :, 16, :], prod[:, :, 17, :])
    tb = prodp.tile([P, C, Wo], f16, tag="tb", name="tb", bufs=1)
    nc.vector.tensor_add(tb[:], t1[:], ta[:])
    tc_ = prodp.tile([P, C, Wo], f16, tag="tc", name="tc_", bufs=1)
    nc.vector.tensor_add(tc_[:], tb[:], prod[:, :, 18, :])
    if first:
        nc.vector.tensor_copy(acc[:], tc_[:])
    else:
        nc.vector.tensor_add(acc[:], acc[:], tc_[:])


def _kh_body(nc, pools, in_e, in_o, acc, w16, kh, first, mode):
    """One kh iteration: mults into prod, kw-reduction, accumulate into acc.

    mode: "red" | "tree" (production) | "multonly" | "red_noacc" (bench-only)."""
    P = RPC
    ring, prodp = pools
    pstep_e = in_e.ap[0][0]

    # All DVE work is engine-serial; single-buffered tiles cost no overlap.
    prod = prodp.tile([P, C, L, Wo], f16, tag="prod", name="prod", bufs=1)
    _mults(nc, in_e, in_o, w16, kh, prod)

    if mode == "multonly":
        return
    if mode in ("red", "red_noacc"):
        # reduce over kw (innermost AP dim), fp32 out
        red_in = _mk(prod, 0, [(prod.ap[0][0], P), (L * Wo, C), (1, Wo), (Wo, L)])
        if first and mode == "red":
            nc.vector.tensor_reduce(
                out=acc[:], in_=red_in, axis=mybir.AxisListType.X, op=mybir.AluOpType.add
            )
        else:
            contrib = ring.tile([P, C, Wo], f32, tag="contrib", name="contrib")
            nc.vector.tensor_reduce(
                out=contrib[:], in_=red_in, axis=mybir.AxisListType.X, op=mybir.AluOpType.add
            )
            if mode == "red":
                nc.vector.tensor_add(acc[:], acc[:], contrib[:])
    elif mode == "tree":
        _tree_fold(nc, prodp, prod, acc, first)
    else:
        raise ValueError(mode)


def _emit(nc, tc, in_d, w_d, out_d, repeat=1, mode=MODE):
    P = RPC
    with (
        tc.tile_pool(name="persist", bufs=1) as persist,
        tc.tile_pool(name="ring", bufs=2) as ring,
        tc.tile_pool(name="prodp", bufs=2) as prodp,
    ):
        in_e = persist.tile([P, L, C, W], f16)      # in_e[i,kh,c,t] = in[c, i+kh, t]
        in_o = persist.tile([P, L, C, W], f16)      # in_o[i,kh,c,t] = in[c, i+kh, t+1]
        acc = persist.tile([P, C, Wo], f32)
        w16_rep = None
        if repeat > 1:
            w16_rep = persist.tile([P, L, Wo], f16)

        _load_input(nc, ring, in_d, in_e, in_o)
        if mode in ("multonly", "red_noacc"):
            nc.vector.memzero(acc[:])  # bench modes never write acc; out-DMA reads it

        def get_w16(kh, rep):
            if rep > 0:
                return w16_rep
            w32 = ring.tile([P, L, Wo], f32, tag="w32", name="w32")
            nc.sync.dma_start(out=w32, in_=w_d[kh])
            w16 = ring.tile([P, L, Wo], f16, tag="w16", name="w16")
            nc.scalar.copy(out=w16, in_=w32)
            if kh == 0 and w16_rep is not None:
                nc.scalar.copy(out=w16_rep[:], in_=w32)
            return w16

        for rep in range(repeat):
            if mode in ("tree2", "tree4"):
                G = 2 if mode == "tree2" else 4
                for kh0 in range(0, L, G):
                    prod_a = prodp.tile([P, C, L, Wo], f16, tag="prodA", name="prod_a", bufs=1)
                    _mults(nc, in_e, in_o, get_w16(kh0, rep), kh0, prod_a)
                    for kh in range(kh0 + 1, min(kh0 + G, L)):
                        prod_b = prodp.tile(
                            [P, C, L, Wo], f16, tag="prodB", name="prod_b", bufs=1
                        )
                        _mults(nc, in_e, in_o, get_w16(kh, rep), kh, prod_b)
                        nc.vector.tensor_add(prod_a[:], prod_a[:], prod_b[:])
                    _tree_fold(nc, prodp, prod_a, acc, first=(kh0 == 0 and rep == 0))
            else:
                for kh in range(L):
                    _kh_body(
                        nc, (ring, prodp), in_e, in_o, acc, get_w16(kh, rep), kh,
                        first=(kh == 0 and rep == 0), mode=mode,
                    )

        nc.sync.dma_start(out=out_d.transpose([1, 0, 2]), in_=acc[:])


def _emit_hw_loop(nc, tc, in_d, w_d, out_d, n_iters, mode=MODE):
    """Timing variant: the exact production main loop repeated n_iters times in
    a hardware loop (values after iteration 0 are garbage; per-iteration
    instruction/DMA mix identical to production)."""
    P = RPC
    with (
        tc.tile_pool(name="persist", bufs=1) as persist,
        tc.tile_pool(name="ring", bufs=2) as ring,
        tc.tile_pool(name="prodp", bufs=2) as prodp,
    ):
        in_e = persist.tile([P, L, C, W], f16)
        in_o = persist.tile([P, L, C, W], f16)
        acc = persist.tile([P, C, Wo], f32)

        _load_input(nc, ring, in_d, in_e, in_o)
        if mode in ("multonly", "red_noacc"):
            nc.vector.memzero(acc[:])  # bench modes never write acc; out-DMA reads it

        def get_w16(kh):
            w32 = ring.tile([P, L, Wo], f32, tag="w32", name="w32")
            nc.sync.dma_start(out=w32, in_=w_d[kh])
            w16 = ring.tile([P, L, Wo], f16, tag="w16", name="w16")
            nc.scalar.copy(out=w16, in_=w32)
            return w16

        with tc.For_i(0, n_iters, 1):
            if mode == "tree2":
                for kh0 in range(0, L, 2):
                    prod_a = prodp.tile([P, C, L, Wo], f16, tag="prodA", name="prod_a", bufs=1)
                    _mults(nc, in_e, in_o, get_w16(kh0), kh0, prod_a)
                    if kh0 + 1 < L:
                        prod_b = prodp.tile(
                            [P, C, L, Wo], f16, tag="prodB", name="prod_b", bufs=1
                        )
                        _mults(nc, in_e, in_o, get_w16(kh0 + 1), kh0 + 1, prod_b)
                        nc.vector.tensor_add(prod_a[:], prod_a[:], prod_b[:])
                    _tree_fold(nc, prodp, prod_a, acc, first=(kh0 == 0))
            else:
                for kh in range(L):
                    _kh_body(
                        nc, (ring, prodp), in_e, in_o, acc, get_w16(kh), kh,
                        first=(kh == 0), mode=mode,
                    )

        nc.sync.dma_start(out=out_d.transpose([1, 0, 2]), in_=acc[:])


def _emit_probe(nc, tc, n_iters, probe):
    """Microbenchmark: 8 identical DVE instructions per hw-loop iteration."""
    P = RPC
    NEL = C * L * Wo  # 13566
    bf16 = mybir.dt.bfloat16
    with (
        tc.tile_pool(name="persist", bufs=1) as persist,
    ):
        dt_map = {"16": f16, "bf": bf16, "32": f32}
        a16 = persist.tile([P, NEL], f16)
        b16 = persist.tile([P, NEL], f16)
        o16 = persist.tile([P, NEL], f16)
        abf = persist.tile([P, NEL], bf16)
        bbf = persist.tile([P, NEL], bf16)
        obf = persist.tile([P, NEL], bf16)
        for t in (a16, b16, abf, bbf):
            nc.vector.memzero(t[:])

        ine = persist.tile([P, C, W], f16)   # small input plane for windowed probes
        wt = persist.tile([P, L, Wo], f16)
        nc.vector.memzero(ine[:])
        nc.vector.memzero(wt[:])

        def win_mult(suffix_dims_a, dims_w, dims_o):
            nc.vector.tensor_mul(
                _mk(o16, 0, dims_o), _mk(ine, 0, suffix_dims_a), _mk(wt, 0, dims_w)
            )

        with tc.For_i(0, n_iters, 1):
            if probe == "tiny":
                # near-empty body: measures the For_i back-edge cost
                nc.vector.tensor_copy(o16[:, :16], a16[:, :16])
            for _ in range(0 if probe == "tiny" else 8):
                if probe == "flat16":
                    nc.vector.tensor_mul(o16[:], a16[:], b16[:])
                elif probe == "flatbf":
                    nc.vector.tensor_mul(obf[:], abf[:], bbf[:])
                elif probe == "flat16_half":
                    nc.vector.tensor_mul(
                        o16[:, : NEL // 2], a16[:, : NEL // 2], b16[:, : NEL // 2]
                    )
                elif probe == "copy16":
                    nc.vector.tensor_copy(o16[:], a16[:])
                elif probe == "mult_noc":
                    # per-c windowed mult, 3-dim APs, no broadcast (KE evens only)
                    for c in range(C):
                        win_mult(
                            [(ine.ap[0][0], P), (2, KE), (1, Wo)],
                            [(wt.ap[0][0], P), (2 * Wo, KE), (1, Wo)],
                            [(o16.ap[0][0], P), (2 * Wo, KE), (1, Wo)],
                        )
                elif probe == "mult_nowin":
                    # same shape but NON-overlapping strided reads from big tiles
                    for c in range(C):
                        nc.vector.tensor_mul(
                            _mk(o16, 0, [(o16.ap[0][0], P), (2 * Wo, KE), (1, Wo)]),
                            _mk(a16, 0, [(a16.ap[0][0], P), (Wo, KE), (1, Wo)]),
                            _mk(b16, 0, [(b16.ap[0][0], P), (2 * Wo, KE), (1, Wo)]),
                        )
                elif probe == "mult_bcast":
                    # the real even-mult shape incl c-broadcast on w (4-dim)
                    nc.vector.tensor_mul(
                        _mk(o16, 0, [(o16.ap[0][0], P), (L * Wo, C), (2 * Wo, KE), (1, Wo)]),
                        _mk(ine, 0, [(ine.ap[0][0], P), (W, C), (2, KE), (1, Wo)]),
                        _mk(wt, 0, [(wt.ap[0][0], P), (0, C), (2 * Wo, KE), (1, Wo)]),
                    )
                else:
                    raise ValueError(probe)


def build_probe(probe, hw_loop):
    key = ("probe", probe, hw_loop)
    with _lock:
        if key in _cache:
            return _cache[key]
        nc = bacc.Bacc("TRN2", target_bir_lowering=False, debug=False)
        in_d = nc.dram_tensor("in_slab", [C, IN_ROWS, W], f32, kind="ExternalInput")
        w_d = nc.dram_tensor("w_slab", [L, RPC, L, Wo], f32, kind="ExternalInput")
        out_d = nc.dram_tensor("out", [C, RPC, Wo], f32, kind="ExternalOutput")
        with tile.TileContext(nc) as tc:
            with tc.tile_pool(name="io", bufs=1) as io:
                sink = io.tile([RPC, C, Wo], f32)
                nc.sync.dma_start(out=sink, in_=w_d[0][:, 0:C, :])
                _emit_probe(nc, tc, hw_loop, probe)
                nc.vector.memzero(sink[:])
                nc.sync.dma_start(out=out_d.transpose([1, 0, 2]), in_=sink[:])
        nc.compile()
        _cache[key] = nc
        return nc
    with _lock:
        if key in _cache:
            return _cache[key]
        nc = bacc.Bacc("TRN2", target_bir_lowering=False, debug=False)
        in_d = nc.dram_tensor("in_slab", [C, IN_ROWS, W], f32, kind="ExternalInput")
        w_d = nc.dram_tensor("w_slab", [L, RPC, L, Wo], f32, kind="ExternalInput")
        out_d = nc.dram_tensor("out", [C, RPC, Wo], f32, kind="ExternalOutput")
        with tile.TileContext(nc) as tc:
            if hw_loop:
                _emit_hw_loop(nc, tc, in_d.ap(), w_d.ap(), out_d.ap(), n_iters=hw_loop, mode=mode)
            else:
                _emit(nc, tc, in_d.ap(), w_d.ap(), out_d.ap(), repeat=repeat, mode=mode)
        nc.compile()
        _cache[key] = nc
        return nc


def build_program(repeat=1, hw_loop=0, mode=MODE):
    key = ("prog", repeat, hw_loop, mode)
    with _lock:
        if key in _cache:
            return _cache[key]
        nc = bacc.Bacc("TRN2", target_bir_lowering=False, debug=False)
        in_d = nc.dram_tensor("in_slab", [C, IN_ROWS, W], f32, kind="ExternalInput")
        w_d = nc.dram_tensor("w_slab", [L, RPC, L, Wo], f32, kind="ExternalInput")
        out_d = nc.dram_tensor("out", [C, RPC, Wo], f32, kind="ExternalOutput")
        with tile.TileContext(nc) as tc:
            if hw_loop:
                _emit_hw_loop(nc, tc, in_d.ap(), w_d.ap(), out_d.ap(), n_iters=hw_loop, mode=mode)
            else:
                _emit(nc, tc, in_d.ap(), w_d.ap(), out_d.ap(), repeat=repeat, mode=mode)
        nc.compile()
        _cache[key] = nc
        return nc


def make_in_maps(input, kernel):
    in_maps = []
    for core in range(NCORES):
        b, half = divmod(core, 2)
        r0 = half * RPC
        in_sl = np.ascontiguousarray(input[b, :, r0 : r0 + IN_ROWS, :], dtype=np.float32)
        kx = kernel[b, :, PAD + r0 : PAD + r0 + RPC, PAD : PAD + Wo]  # [361, 119, 238]
        w_sl = np.ascontiguousarray(
            kx.reshape(L, L, RPC, Wo).transpose(0, 2, 1, 3), dtype=np.float32
        )  # [kh, i, kw, j]
        in_maps.append({"in_slab": in_sl, "w_slab": w_sl})
    return in_maps


def gather_out(results):
    out = np.empty((B, C, Ho, Wo), dtype=np.float32)
    for core in range(NCORES):
        b, half = divmod(core, 2)
        out[b, :, half * RPC : (half + 1) * RPC, :] = results[core]["out"]
    return out


def run(input, kernel, **spmd_kwargs):
    nc = build_program()
    in_maps = make_in_maps(input, kernel)
    res = run_bass_kernel_spmd(nc, in_maps, core_ids=list(range(NCORES)), **spmd_kwargs)
    return gather_out(res.results), res


def kernel(**inputs):
    out, _ = run(np.asarray(inputs["input"]), np.asarray(inputs["kernel"]))
    return out
